# revision 6
# baseline (speedup 1.0000x reference)
"""EulerRotaryAttention Trainium2 kernel (bf16 matmul pipeline), v2.

Sharding: 8 cores = 2 (batch) x 4 (head groups of 4 heads).  Each core
computes the qkv projection for its heads, rotary attention, and a partial
o-projection; the host sums partials over the 4 head groups per batch.

Device dataflow (zero on-device transposes):
  - x^T arrives pre-transposed from the host as (d, n), bf16.
  - Q^T, K^T computed directly in (feat, tok) layout with the projection
    weights as the stationary matmul operand; fp32 PSUM accumulation.
  - RoPE rotation applied during PSUM eviction (host-permuted feature
    pairs 32 partitions apart; host-precomputed cos/sin tables).
  - S^T in (k, q) layout, causal tiles only, paired heads run as
    concurrent 64-row PE tiles; exp on ScalarE (no max subtraction).
  - PV: lhsT = [V | 1] so the accumulator yields A^T and the softmax
    denominators; reciprocal via the fast custom-DVE approx directly on
    the PSUM denominator row; partition broadcast via a DRAM bounce.
  - o-projection consumes A^T as lhsT; partial (n, d) written per core
    and summed on the host.

v2 schedule (vs. baseline): the whole kernel runs in ONE pool scope with
three PSUM pools (wide 2x2-bank slots shared by qkv-projection groups and
S-tiles, narrow 2x1-bank shared by V and PV, o 2x1-bank).  The second
token-half of the qkv projection and V tiles 8-15 are woven between the
early attention iterations, so ScalarE exp work starts ~30us earlier.
Eviction work is split Scalar/Vector by phase so exp is never queued
behind eviction copies.
"""

import math

import numpy as np

B, N, D, H = 2, 2048, 1024, 16
DH = D // H  # 64
HL = 4  # local heads per core
DL = HL * DH  # 256 local features
KC = D // 128  # 8 contraction chunks
NT = N // 128  # 16 token tiles
NCORES = 8

EULER_BASIS = (1.0, math.pi, math.e, math.pi * math.e, math.pi / math.e)

_PROG = None
LAST_RESULTS = None


def _build_program():
    import concourse.bass as bass
    import concourse.mybir as mybir
    import concourse.tile as tile
    from concourse import bacc

    f32 = mybir.dt.float32
    bf = mybir.dt.bfloat16
    AF = mybir.ActivationFunctionType

    nc = bacc.Bacc("TRN2", target_bir_lowering=False, num_devices=NCORES)

    xT = nc.declare_dram_parameter("xT", [128, KC, N], bf, isOutput=False)
    wq = nc.declare_dram_parameter("wq", [128, KC, DL], bf, isOutput=False)
    wk = nc.declare_dram_parameter("wk", [128, KC, DL], bf, isOutput=False)
    wv = nc.declare_dram_parameter("wv", [128, KC, DL], bf, isOutput=False)
    wo = nc.declare_dram_parameter("wo", [128, 2, D], bf, isOutput=False)
    ctab = nc.declare_dram_parameter("ctab", [128, 2, N], bf, isOutput=False)
    stab = nc.declare_dram_parameter("stab", [128, 2, N], bf, isOutput=False)
    tri8 = nc.declare_dram_parameter("tri8", [128, 8, 128], bf, isOutput=False)
    o_out = nc.declare_dram_parameter("o_out", [NT, 128, D], bf, isOutput=True)

    with tile.TileContext(nc) as tc:
        with (
            tc.tile_pool(name="persist", bufs=1) as persist,
            tc.tile_pool(name="exps_pool", bufs=2) as exps_pool,
            tc.tile_pool(name="rot_tmp", bufs=2) as rot_tmp,
            tc.tile_pool(name="norm_pool", bufs=2) as norm_pool,
            tc.tile_pool(name="bcast_pool", bufs=2) as bcast_pool,
            tc.tile_pool(name="dscr_pool", bufs=4, space="DRAM") as dscr_pool,
            tc.tile_pool(name="ostage_pool", bufs=3) as ostage_pool,
            # PSUM: 2x 2-bank slots (qkv-proj groups + S tiles), 2x 1-bank
            # (V + PV tiles), 2x 1-bank (o tiles) = 8 banks total
            tc.tile_pool(name="wide", bufs=2, space="PSUM") as wide,
            tc.tile_pool(name="narrow", bufs=2, space="PSUM") as narrow,
            tc.tile_pool(name="opsum_pool", bufs=2, space="PSUM") as opsum_pool,
        ):
            # rotated Q^T / K^T: (256 feats, N) as 2 x (128, N), bf16
            qt_rot = [
                persist.tile([128, N], bf, tag=f"qt{m}", name=f"qt{m}")
                for m in range(2)
            ]
            kt_rot = [
                persist.tile([128, N], bf, tag=f"kt{m}", name=f"kt{m}")
                for m in range(2)
            ]
            # V for all heads with appended ones column: (128, NT, HL, 65)
            vones = persist.tile([128, NT, HL, DH + 1], bf, tag="vones", name="vones")
            nc.vector.memset(vones[:, :, :, DH : DH + 1], 1.0)
            # A^T head pairs: (128, N) bf16
            at2 = [
                persist.tile([128, N], bf, tag=f"at{m}", name=f"at{m}")
                for m in range(2)
            ]
            xT_sb = persist.tile([128, KC, N], bf, tag="xT", name="xT_sb")
            wv_sb = persist.tile([128, KC, DL], bf, tag="wv", name="wv_sb")
            wq_sb = persist.tile([128, KC, DL], bf, tag="wq", name="wq_sb")
            wk_sb = persist.tile([128, KC, DL], bf, tag="wk", name="wk_sb")
            ctab_sb = persist.tile([128, 2, N], bf, tag="ctab", name="ctab_sb")
            stab_sb = persist.tile([128, 2, N], bf, tag="stab", name="stab_sb")
            tri8_sb = persist.tile([128, 8, 128], bf, tag="tri8", name="tri8_sb")
            wo_sb = persist.tile([128, 2, D], bf, tag="wo", name="wo_sb")

            # DMA order = first-use order.  xT split per (kc, token-half) so
            # the first projection group is not gated on the full 4MB.
            nc.sync.dma_start(out=wk_sb[:], in_=wk[:])
            for kc in range(KC):
                nc.sync.dma_start(
                    out=xT_sb[:, kc, 0:1024], in_=xT[:, kc, 0:1024]
                )
            nc.sync.dma_start(out=wq_sb[:], in_=wq[:])
            nc.sync.dma_start(out=ctab_sb[:], in_=ctab[:])
            nc.sync.dma_start(out=stab_sb[:], in_=stab[:])
            nc.sync.dma_start(out=tri8_sb[:], in_=tri8[:])
            for kc in range(KC):
                nc.sync.dma_start(
                    out=xT_sb[:, kc, 1024:2048], in_=xT[:, kc, 1024:2048]
                )
            nc.sync.dma_start(out=wv_sb[:], in_=wv[:])
            nc.sync.dma_start(out=wo_sb[:], in_=wo[:])

            def qkt_group(w_sb, rot, mt, nh, raw_on_scalar):
                """One (weight, head-pair, token-half) projection group with
                fused rotation eviction: rot = raw*ctab + swap32(raw)*stab."""
                nsl = slice(nh * 1024, (nh + 1) * 1024)
                psum = wide.tile([128, 1024], f32, tag="wide", name="qkpsum")
                for kc in range(KC):
                    for nq in range(2):
                        nc.tensor.matmul(
                            psum[:, nq * 512 : (nq + 1) * 512],
                            w_sb[:, kc, mt * 128 : (mt + 1) * 128],
                            xT_sb[
                                :,
                                kc,
                                nh * 1024 + nq * 512 : nh * 1024 + (nq + 1) * 512,
                            ],
                            start=(kc == 0),
                            stop=(kc == KC - 1),
                        )
                raw = rot_tmp.tile([128, 1024], bf, tag="raw", name="raw")
                if raw_on_scalar:
                    nc.scalar.copy(out=raw[:], in_=psum[:])
                else:
                    nc.vector.tensor_copy(out=raw[:], in_=psum[:])
                nc.vector.tensor_mul(rot[mt][:, nsl], raw[:], ctab_sb[:, mt, nsl])
                raws = rot_tmp.tile([128, 1024], bf, tag="rs", name="raws")
                for g in range(4):
                    s = g ^ 1
                    nc.vector.tensor_copy(
                        raws[g * 32 : (g + 1) * 32, :],
                        raw[s * 32 : (s + 1) * 32, :],
                    )
                tmp = rot_tmp.tile([128, 1024], bf, tag="rt", name="tmp")
                nc.vector.tensor_mul(tmp[:], raws[:], stab_sb[:, mt, nsl])
                nc.vector.tensor_add(rot[mt][:, nsl], rot[mt][:, nsl], tmp[:])

            def v_proj(tts, evict_on_scalar):
                for tt in tts:
                    vpsum = narrow.tile([128, DL], f32, tag="narrow", name="vpsum")
                    for kc in range(KC):
                        nc.tensor.matmul(
                            vpsum[:],
                            xT_sb[:, kc, tt * 128 : (tt + 1) * 128],
                            wv_sb[:, kc, :],
                            start=(kc == 0),
                            stop=(kc == KC - 1),
                        )
                    src = vpsum[:].rearrange("p (h d) -> p h d", h=HL)
                    if evict_on_scalar:
                        nc.scalar.copy(out=vones[:, tt, :, 0:DH], in_=src)
                    else:
                        nc.vector.tensor_copy(out=vones[:, tt, :, 0:DH], in_=src)

            rcp4s = {}

            def emit_s_exp(qc, mt):
                # paired-head S matmuls: the two heads of pair `mt` sit in
                # disjoint 64-partition halves of qt/kt -> concurrent PE
                # row-tiles; one ACT exp covers the pair
                nkt = 4 * qc + 4
                exps = exps_pool.tile([128, NT, 2, 512], bf, tag="e", name="exps")
                for kt in range(nkt):
                    j = kt - 4 * qc
                    jo = max(j, 0) * 128
                    spsum = wide.tile([128, 2, 512], f32, tag="wide", name="spsum")
                    for eo in range(2):
                        roff = eo * 64
                        nc.tensor.matmul(
                            spsum[:, eo, jo:512],
                            kt_rot[mt][roff : roff + 64, kt * 128 : (kt + 1) * 128],
                            qt_rot[mt][
                                roff : roff + 64,
                                qc * 512 + jo : qc * 512 + 512,
                            ],
                            start=True,
                            stop=True,
                        )
                    nc.scalar.activation(
                        exps[:, kt, :, jo:512], spsum[:, :, jo:512], AF.Exp
                    )
                # mask the 4 diagonal 128x128 subtiles of BOTH heads in one
                # strided op: (p, j, eo, c) -> exps[p, 4*qc+j, eo, j*128+c]
                sub = exps[:, 4 * qc, 0, :]
                diag = bass.AP(
                    tensor=sub.tensor,
                    offset=sub.offset,
                    ap=[list(sub.ap[0]), [1152, 4], [512, 2], [1, 128]],
                )
                t8 = tri8_sb[:, 0, :]
                trip = bass.AP(
                    tensor=t8.tensor,
                    offset=t8.offset,
                    ap=[list(t8.ap[0]), [256, 4], [128, 2], [1, 128]],
                )
                nc.vector.tensor_mul(diag, diag, trip)
                return exps

            def emit_pv_evict(qc, mt, exps):
                qsl = slice(qc * 512, (qc + 1) * 512)
                nkt = 4 * qc + 4
                if mt == 0:
                    # denominator rows live at partitions 0/32/64/96 (the
                    # only legal engine start partitions); unused rows are
                    # memset to 1.0 so the batched reciprocal stays finite
                    rcp4s[qc] = norm_pool.tile([97, 512], f32, tag="rcp", name="rcp4")
                    nc.gpsimd.memset(rcp4s[qc][:], 1.0)
                rcp4 = rcp4s[qc]
                for eo in range(2):
                    h = 2 * mt + eo
                    roff = eo * 64
                    pv = narrow.tile([DH + 1, 512], f32, tag="narrow", name="pv")
                    for kt in range(nkt):
                        j = kt - 4 * qc
                        jo = max(j, 0) * 128
                        nc.tensor.matmul(
                            pv[:, jo:512],
                            vones[:, kt, h, :],
                            exps[:, kt, eo, jo:512],
                            start=(kt == 0),
                            stop=(kt == nkt - 1),
                        )
                    nc.vector.tensor_copy(
                        out=rcp4[32 * h : 32 * h + 1, :], in_=pv[DH : DH + 1, :]
                    )
                    # evict unnormalized A^T
                    if qc == 3 and mt == 1:
                        nc.scalar.copy(
                            out=at2[mt][roff : roff + DH, qsl], in_=pv[0:DH, :]
                        )
                    else:
                        nc.vector.tensor_copy(
                            out=at2[mt][roff : roff + DH, qsl], in_=pv[0:DH, :]
                        )

            def emit_normalize(qc):
                qsl = slice(qc * 512, (qc + 1) * 512)
                dnm4 = rcp4s[qc]
                rcp4 = norm_pool.tile([97, 512], f32, tag="rcpo", name="rcp4o")
                nc.vector.reciprocal(rcp4[:], dnm4[:])
                for mt in range(2):
                    bc = bcast_pool.tile([128, 512], f32, tag="bc", name="bc")
                    # broadcast each head's reciprocal row across 64
                    # partitions: bounce through DRAM, then a step-0
                    # partition DMA (legal for DRAM sources only)
                    for half in range(2):
                        row = rcp4[64 * mt + 32 * half : 64 * mt + 32 * half + 1, :]
                        rdram = dscr_pool.tile([1, 512], f32, tag="rd", name="rd")
                        nc.gpsimd.dma_start(out=rdram[:], in_=row)
                        rd = rdram[:]
                        nc.gpsimd.dma_start(
                            out=bc[64 * half : 64 * half + 64, :],
                            in_=bass.AP(
                                tensor=rd.tensor,
                                offset=rd.offset,
                                ap=[[0, 64], [1, 512]],
                            ),
                        )
                    nc.vector.tensor_mul(at2[mt][:, qsl], at2[mt][:, qsl], bc[:])

            def o_proj_block(qc):
                for tt in range(4 * qc, 4 * qc + 4):
                    ost = ostage_pool.tile([128, D], bf, tag="ost", name="ost")
                    for nb in range(2):
                        opsum = opsum_pool.tile(
                            [128, 512], f32, tag="o", name="opsum"
                        )
                        for hp in range(2):
                            nc.tensor.matmul(
                                opsum[:],
                                at2[hp][:, tt * 128 : (tt + 1) * 128],
                                wo_sb[:, hp, nb * 512 : (nb + 1) * 512],
                                start=(hp == 0),
                                stop=(hp == 1),
                            )
                        dst = ost[:, nb * 512 : (nb + 1) * 512]
                        if qc == 3:
                            nc.scalar.copy(out=dst, in_=opsum[:])
                        else:
                            nc.vector.tensor_copy(out=dst, in_=opsum[:])
                    nc.sync.dma_start(out=o_out[tt], in_=ost[:])

            # ============ emission schedule ============
            # first token-half of the projections, then V tiles 0-7
            qkt_group(wk_sb, kt_rot, 0, 0, raw_on_scalar=True)
            qkt_group(wk_sb, kt_rot, 1, 0, raw_on_scalar=True)
            qkt_group(wq_sb, qt_rot, 0, 0, raw_on_scalar=True)
            qkt_group(wq_sb, qt_rot, 1, 0, raw_on_scalar=True)
            v_proj(range(0, 4), evict_on_scalar=True)
            v_proj(range(4, 8), evict_on_scalar=True)

            # second token-half + V 8-15 woven between attention iterations
            OVERLAP = True
            fillers = {
                (0, 0): lambda: qkt_group(wk_sb, kt_rot, 0, 1, raw_on_scalar=False),
                (0, 1): lambda: qkt_group(wk_sb, kt_rot, 1, 1, raw_on_scalar=False),
                (1, 0): lambda: qkt_group(wq_sb, qt_rot, 0, 1, raw_on_scalar=False),
                (1, 1): lambda: qkt_group(wq_sb, qt_rot, 1, 1, raw_on_scalar=False),
                (2, 0): lambda: (
                    v_proj(range(8, 12), evict_on_scalar=False),
                    v_proj(range(12, 16), evict_on_scalar=False),
                ),
            }
            if not OVERLAP:
                for key in sorted(fillers):
                    fillers[key]()
                fillers = {}

            iters = [(qc, mt) for qc in range(4) for mt in range(2)]
            pending = None  # (qc, mt, exps)
            for qc, mt in iters:
                exps = emit_s_exp(qc, mt)
                if pending is not None:
                    emit_pv_evict(*pending)
                    if pending[1] == 1:
                        emit_normalize(pending[0])
                        o_proj_block(pending[0])
                if (qc, mt) in fillers:
                    fillers[(qc, mt)]()
                pending = (qc, mt, exps)
            emit_pv_evict(*pending)
            emit_normalize(pending[0])
            o_proj_block(pending[0])

    nc.compile()
    return nc


def get_program():
    global _PROG
    if _PROG is None:
        _PROG = _build_program()
    return _PROG


def _host_tables(bit_logits):
    """Replicate the reference fp32 cos/sin computation exactly (jax on CPU)."""
    import jax

    with jax.default_device(jax.devices("cpu")[0]):
        import jax.numpy as jnp

        basis = jnp.asarray(EULER_BASIS, dtype=jnp.float32)
        freqs = jax.nn.sigmoid(jnp.asarray(bit_logits, dtype=jnp.float32)) @ basis
        inv_freq = 2.0 ** (-(jnp.arange(0, DH, 2, dtype=jnp.float32) / DH))
        pos = jnp.arange(N, dtype=jnp.float32)
        theta = pos[None, :, None] * freqs[:, None, None] * inv_freq[None, None, :]
        cos = np.asarray(jnp.cos(theta))  # (H, N, 32)
        sin = np.asarray(jnp.sin(theta))
    return cos, sin


def _chunk_rows(a, p=128):
    """(R, C) -> (p, R//p, C); row r = kc*p + pp lands at [pp, kc]."""
    r, c = a.shape
    return np.ascontiguousarray(a.reshape(r // p, p, c).transpose(1, 0, 2))


def prepare_inputs(x, w_qkv, w_o, bit_logits):
    import ml_dtypes

    bf = ml_dtypes.bfloat16

    x = np.asarray(x, dtype=np.float32)
    w_qkv = np.asarray(w_qkv, dtype=np.float32)
    w_o = np.asarray(w_o, dtype=np.float32)
    cos, sin = _host_tables(np.asarray(bit_logits, dtype=np.float32))

    # de-interleave permutation within a head: evens then odds
    perm = np.concatenate([np.arange(0, DH, 2), np.arange(1, DH, 2)])

    wq_full = w_qkv.reshape(D, 3, H, DH)[:, 0]  # (D, H, DH)
    wk_full = w_qkv.reshape(D, 3, H, DH)[:, 1]
    wv_full = w_qkv.reshape(D, 3, H, DH)[:, 2]
    scale = 1.0 / math.sqrt(DH)

    # tri[krow, qcol] = 1 if qcol >= krow else 0, replicated 8x for the
    # strided diagonal mask
    tri = np.triu(np.ones((128, 128), dtype=np.float32))
    tri8 = np.broadcast_to(tri[:, None, :], (128, 8, 128)).copy()

    xT_by_batch = [
        _chunk_rows(np.ascontiguousarray(x[b].T)) for b in range(B)
    ]  # (128, KC, N)

    per_group = []
    for g in range(4):
        heads = range(4 * g, 4 * g + 4)
        wq_g = np.concatenate(
            [wq_full[:, h][:, perm] * scale for h in heads], axis=1
        )  # (D, 256)
        wk_g = np.concatenate([wk_full[:, h][:, perm] for h in heads], axis=1)
        wv_g = np.concatenate([wv_full[:, h] for h in heads], axis=1)
        wo_g = np.concatenate(
            [w_o.reshape(H, DH, D)[h] for h in heads], axis=0
        )  # (256, D)

        # rotation tables, layout (256 feats, N) -> (128, 2, N)
        ct = np.empty((DL, N), dtype=np.float32)
        st = np.empty((DL, N), dtype=np.float32)
        for hl, h in enumerate(heads):
            c = cos[h].T  # (32, N)
            s = sin[h].T
            ct[hl * DH : hl * DH + 32] = c
            ct[hl * DH + 32 : hl * DH + 64] = c
            st[hl * DH : hl * DH + 32] = -s
            st[hl * DH + 32 : hl * DH + 64] = s
        per_group.append(
            dict(
                wq=_chunk_rows(wq_g).astype(bf),
                wk=_chunk_rows(wk_g).astype(bf),
                wv=_chunk_rows(wv_g).astype(bf),
                wo=_chunk_rows(wo_g).astype(bf),
                ctab=_chunk_rows(ct).astype(bf),
                stab=_chunk_rows(st).astype(bf),
                tri8=tri8.astype(bf),
            )
        )

    in_maps = []
    for c in range(NCORES):
        b, g = c // 4, c % 4
        m = dict(per_group[g])
        m["xT"] = xT_by_batch[b].astype(bf)
        in_maps.append(m)
    return in_maps


def kernel(x, w_qkv, w_o, bit_logits, n_heads):
    global LAST_RESULTS
    from concourse.bass_utils import run_bass_kernel_spmd

    assert int(n_heads) == H
    nc = get_program()
    in_maps = prepare_inputs(x, w_qkv, w_o, bit_logits)
    res = run_bass_kernel_spmd(nc, in_maps, list(range(NCORES)))
    LAST_RESULTS = res
    out = np.zeros((B, N, D), dtype=np.float32)
    for c in range(NCORES):
        b = c // 4
        out[b] += res.results[c]["o_out"].reshape(N, D).astype(np.float32)
    return out


# revision 11
# speedup vs baseline: 1.1852x; 1.1852x over previous
"""EulerRotaryAttention Trainium2 kernel (bf16 matmul pipeline), v2.

Sharding: 8 cores = 2 (batch) x 4 (head groups of 4 heads).  Each core
computes the qkv projection for its heads, rotary attention, and a partial
o-projection; the host sums partials over the 4 head groups per batch.

Device dataflow (zero on-device transposes):
  - x^T arrives pre-transposed from the host as (d, n), bf16.
  - Q^T, K^T computed directly in (feat, tok) layout with the projection
    weights as the stationary matmul operand; fp32 PSUM accumulation.
  - RoPE rotation applied during PSUM eviction (host-permuted feature
    pairs 32 partitions apart; host-precomputed cos/sin tables).
  - S^T in (k, q) layout, causal tiles only, paired heads run as
    concurrent 64-row PE tiles; exp on ScalarE (no max subtraction).
  - PV: lhsT = [V | 1] so the accumulator yields A^T and the softmax
    denominators; reciprocal via the fast custom-DVE approx directly on
    the PSUM denominator row; partition broadcast via a DRAM bounce.
  - o-projection consumes A^T as lhsT; partial (n, d) written per core
    and summed on the host.

v2 schedule (vs. baseline): the whole kernel runs in ONE pool scope with
three PSUM pools (wide 2x2-bank slots shared by qkv-projection groups and
S-tiles, narrow 2x1-bank shared by V and PV, o 2x1-bank).  The second
token-half of the qkv projection and V tiles 8-15 are woven between the
early attention iterations, so ScalarE exp work starts ~30us earlier.
Eviction work is split Scalar/Vector by phase so exp is never queued
behind eviction copies.
"""

import math

import numpy as np

B, N, D, H = 2, 2048, 1024, 16
DH = D // H  # 64
HL = 4  # local heads per core
DL = HL * DH  # 256 local features
KC = D // 128  # 8 contraction chunks
NT = N // 128  # 16 token tiles
NCORES = 8

EULER_BASIS = (1.0, math.pi, math.e, math.pi * math.e, math.pi / math.e)

_PROG = None
LAST_RESULTS = None


def _build_program():
    import concourse.bass as bass
    import concourse.mybir as mybir
    import concourse.tile as tile
    from concourse import bacc

    f32 = mybir.dt.float32
    bf = mybir.dt.bfloat16
    AF = mybir.ActivationFunctionType

    nc = bacc.Bacc("TRN2", target_bir_lowering=False, num_devices=NCORES)

    xT = nc.declare_dram_parameter("xT", [128, KC, N], bf, isOutput=False)
    wq = nc.declare_dram_parameter("wq", [128, KC, DL], bf, isOutput=False)
    wk = nc.declare_dram_parameter("wk", [128, KC, DL], bf, isOutput=False)
    wv = nc.declare_dram_parameter("wv", [128, KC, DL], bf, isOutput=False)
    wo = nc.declare_dram_parameter("wo", [128, 2, D], bf, isOutput=False)
    ctab = nc.declare_dram_parameter("ctab", [128, 2, N], bf, isOutput=False)
    stab = nc.declare_dram_parameter("stab", [128, 2, N], bf, isOutput=False)
    tri8 = nc.declare_dram_parameter("tri8", [128, 8, 128], bf, isOutput=False)
    o_out = nc.declare_dram_parameter("o_out", [NT, 128, D], bf, isOutput=True)

    with tile.TileContext(nc) as tc:
        with (
            tc.tile_pool(name="persist", bufs=1) as persist,
            tc.tile_pool(name="exps_pool", bufs=2) as exps_pool,
            tc.tile_pool(name="rot_tmp", bufs=2) as rot_tmp,
            tc.tile_pool(name="norm_pool", bufs=2) as norm_pool,
            tc.tile_pool(name="bcast_pool", bufs=2) as bcast_pool,
            tc.tile_pool(name="dscr_pool", bufs=4, space="DRAM") as dscr_pool,
            tc.tile_pool(name="ostage_pool", bufs=3) as ostage_pool,
            # PSUM: 2x 2-bank slots (qkv-proj groups + S tiles), 2x 1-bank
            # (V + PV tiles), 2x 1-bank (o tiles) = 8 banks total
            tc.tile_pool(name="wide", bufs=2, space="PSUM") as wide,
            tc.tile_pool(name="narrow", bufs=2, space="PSUM") as narrow,
            tc.tile_pool(name="opsum_pool", bufs=2, space="PSUM") as opsum_pool,
        ):
            # rotated Q^T / K^T: (256 feats, N) as 2 x (128, N), bf16
            qt_rot = [
                persist.tile([128, N], bf, tag=f"qt{m}", name=f"qt{m}")
                for m in range(2)
            ]
            kt_rot = [
                persist.tile([128, N], bf, tag=f"kt{m}", name=f"kt{m}")
                for m in range(2)
            ]
            # V for all heads with appended ones column: (128, NT, HL, 65)
            vones = persist.tile([128, NT, HL, DH + 1], bf, tag="vones", name="vones")
            nc.vector.memset(vones[:, :, :, DH : DH + 1], 1.0)
            # A^T head pairs: (128, N) bf16
            at2 = [
                persist.tile([128, N], bf, tag=f"at{m}", name=f"at{m}")
                for m in range(2)
            ]
            xT_sb = persist.tile([128, KC, N], bf, tag="xT", name="xT_sb")
            wv_sb = persist.tile([128, KC, DL], bf, tag="wv", name="wv_sb")
            wq_sb = persist.tile([128, KC, DL], bf, tag="wq", name="wq_sb")
            wk_sb = persist.tile([128, KC, DL], bf, tag="wk", name="wk_sb")
            ctab_sb = persist.tile([128, 2, N], bf, tag="ctab", name="ctab_sb")
            stab_sb = persist.tile([128, 2, N], bf, tag="stab", name="stab_sb")
            tri8_sb = persist.tile([128, 8, 128], bf, tag="tri8", name="tri8_sb")
            wo_sb = persist.tile([128, 2, D], bf, tag="wo", name="wo_sb")

            # DMA order = first-use order.  xT split per (kc, token-half) so
            # the first projection group is not gated on the full 4MB.
            nc.sync.dma_start(out=wk_sb[:], in_=wk[:])
            for kc in range(KC):
                nc.sync.dma_start(
                    out=xT_sb[:, kc, 0:1024], in_=xT[:, kc, 0:1024]
                )
            nc.sync.dma_start(out=wq_sb[:], in_=wq[:])
            nc.sync.dma_start(out=ctab_sb[:], in_=ctab[:])
            nc.sync.dma_start(out=stab_sb[:], in_=stab[:])
            nc.sync.dma_start(out=tri8_sb[:], in_=tri8[:])
            for kc in range(KC):
                nc.sync.dma_start(
                    out=xT_sb[:, kc, 1024:2048], in_=xT[:, kc, 1024:2048]
                )
            nc.sync.dma_start(out=wv_sb[:], in_=wv[:])
            nc.sync.dma_start(out=wo_sb[:], in_=wo[:])

            def qkt_group(w_sb, rot, mt, nh, raw_on_scalar):
                """One (weight, head-pair, token-half) projection group with
                fused rotation eviction: rot = raw*ctab + swap32(raw)*stab."""
                nsl = slice(nh * 1024, (nh + 1) * 1024)
                psum = wide.tile([128, 1024], f32, tag="wide", name="qkpsum")
                for kc in range(KC):
                    for nq in range(2):
                        nc.tensor.matmul(
                            psum[:, nq * 512 : (nq + 1) * 512],
                            w_sb[:, kc, mt * 128 : (mt + 1) * 128],
                            xT_sb[
                                :,
                                kc,
                                nh * 1024 + nq * 512 : nh * 1024 + (nq + 1) * 512,
                            ],
                            start=(kc == 0),
                            stop=(kc == KC - 1),
                        )
                raw = rot_tmp.tile([128, 1024], bf, tag="raw", name="raw")
                if raw_on_scalar:
                    nc.scalar.copy(out=raw[:], in_=psum[:])
                else:
                    nc.vector.tensor_copy(out=raw[:], in_=psum[:])
                nc.vector.tensor_mul(rot[mt][:, nsl], raw[:], ctab_sb[:, mt, nsl])
                raws = rot_tmp.tile([128, 1024], bf, tag="rs", name="raws")
                for g in range(4):
                    s = g ^ 1
                    nc.vector.tensor_copy(
                        raws[g * 32 : (g + 1) * 32, :],
                        raw[s * 32 : (s + 1) * 32, :],
                    )
                tmp = rot_tmp.tile([128, 1024], bf, tag="rt", name="tmp")
                nc.vector.tensor_mul(tmp[:], raws[:], stab_sb[:, mt, nsl])
                nc.vector.tensor_add(rot[mt][:, nsl], rot[mt][:, nsl], tmp[:])

            def v_proj(tts, evict_on_scalar):
                for tt in tts:
                    vpsum = narrow.tile([128, DL], f32, tag="narrow", name="vpsum")
                    for kc in range(KC):
                        nc.tensor.matmul(
                            vpsum[:],
                            xT_sb[:, kc, tt * 128 : (tt + 1) * 128],
                            wv_sb[:, kc, :],
                            start=(kc == 0),
                            stop=(kc == KC - 1),
                        )
                    src = vpsum[:].rearrange("p (h d) -> p h d", h=HL)
                    if evict_on_scalar:
                        nc.scalar.copy(out=vones[:, tt, :, 0:DH], in_=src)
                    else:
                        nc.vector.tensor_copy(out=vones[:, tt, :, 0:DH], in_=src)

            rcp4s = {}

            def emit_s_exp(qc, mt):
                # paired-head S matmuls: the two heads of pair `mt` sit in
                # disjoint 64-partition halves of qt/kt -> concurrent PE
                # row-tiles; one ACT exp covers the pair
                nkt = 4 * qc + 4
                exps = exps_pool.tile([128, NT, 2, 512], bf, tag="e", name="exps")
                for kt in range(nkt):
                    j = kt - 4 * qc
                    jo = max(j, 0) * 128
                    spsum = wide.tile([128, 2, 512], f32, tag="wide", name="spsum")
                    for eo in range(2):
                        roff = eo * 64
                        nc.tensor.matmul(
                            spsum[:, eo, jo:512],
                            kt_rot[mt][roff : roff + 64, kt * 128 : (kt + 1) * 128],
                            qt_rot[mt][
                                roff : roff + 64,
                                qc * 512 + jo : qc * 512 + 512,
                            ],
                            start=True,
                            stop=True,
                        )
                    nc.scalar.activation(
                        exps[:, kt, :, jo:512], spsum[:, :, jo:512], AF.Exp
                    )
                # mask the 4 diagonal 128x128 subtiles of BOTH heads in one
                # strided op: (p, j, eo, c) -> exps[p, 4*qc+j, eo, j*128+c]
                sub = exps[:, 4 * qc, 0, :]
                diag = bass.AP(
                    tensor=sub.tensor,
                    offset=sub.offset,
                    ap=[list(sub.ap[0]), [1152, 4], [512, 2], [1, 128]],
                )
                t8 = tri8_sb[:, 0, :]
                trip = bass.AP(
                    tensor=t8.tensor,
                    offset=t8.offset,
                    ap=[list(t8.ap[0]), [256, 4], [128, 2], [1, 128]],
                )
                nc.vector.tensor_mul(diag, diag, trip)
                return exps

            def emit_pv_evict(qc, mt, exps):
                qsl = slice(qc * 512, (qc + 1) * 512)
                nkt = 4 * qc + 4
                if mt == 0:
                    # denominator rows live at partitions 0/32/64/96 (the
                    # only legal engine start partitions); pad rows are
                    # memset so the reciprocal input is fully initialized
                    rcp4s[qc] = norm_pool.tile([97, 512], f32, tag="rcp", name="rcp4")
                    nc.gpsimd.memset(rcp4s[qc][:], 1.0)
                rcp4 = rcp4s[qc]
                for eo in range(2):
                    h = 2 * mt + eo
                    roff = eo * 64
                    pv = narrow.tile([DH + 1, 512], f32, tag="narrow", name="pv")
                    for kt in range(nkt):
                        j = kt - 4 * qc
                        jo = max(j, 0) * 128
                        nc.tensor.matmul(
                            pv[:, jo:512],
                            vones[:, kt, h, :],
                            exps[:, kt, eo, jo:512],
                            start=(kt == 0),
                            stop=(kt == nkt - 1),
                        )
                    nc.vector.tensor_copy(
                        out=rcp4[32 * h : 32 * h + 1, :], in_=pv[DH : DH + 1, :]
                    )
                    # evict unnormalized A^T
                    if qc == 3 and mt == 1:
                        nc.scalar.copy(
                            out=at2[mt][roff : roff + DH, qsl], in_=pv[0:DH, :]
                        )
                    else:
                        nc.vector.tensor_copy(
                            out=at2[mt][roff : roff + DH, qsl], in_=pv[0:DH, :]
                        )

            rcp4os = {}

            def emit_normalize_mt(qc, mt):
                # per head-pair normalization so the mt=0 chain overlaps the
                # mt=1 PV instead of serializing after it
                qsl = slice(qc * 512, (qc + 1) * 512)
                dnm4 = rcp4s[qc]
                if mt == 0:
                    rcp4os[qc] = norm_pool.tile(
                        [97, 512], f32, tag="rcpo", name="rcp4o"
                    )
                rcp4 = rcp4os[qc]
                sl = slice(64 * mt, 64 * mt + 33)
                # 1/d = exp(-ln d) on ScalarE: Ln and Exp share one ACT
                # table set, so this costs no table reloads and stays off
                # the busy VectorE (the custom-DVE reciprocal_approx ops
                # produce garbage on this hardware path)
                lnd = norm_pool.tile([97, 512], f32, tag="lnd", name="lnd")
                nc.scalar.activation(lnd[sl, :], dnm4[sl, :], AF.Ln)
                nc.scalar.activation(rcp4[sl, :], lnd[sl, :], AF.Exp, scale=-1.0)
                bc = bcast_pool.tile([128, 512], f32, tag="bc", name="bc")
                # broadcast each head's reciprocal row across 64 partitions:
                # bounce through DRAM, then a step-0 partition DMA (legal
                # for DRAM sources only)
                for half in range(2):
                    row = rcp4[64 * mt + 32 * half : 64 * mt + 32 * half + 1, :]
                    rdram = dscr_pool.tile([1, 512], f32, tag="rd", name="rd")
                    nc.gpsimd.dma_start(out=rdram[:], in_=row)
                    rd = rdram[:]
                    nc.gpsimd.dma_start(
                        out=bc[64 * half : 64 * half + 64, :],
                        in_=bass.AP(
                            tensor=rd.tensor,
                            offset=rd.offset,
                            ap=[[0, 64], [1, 512]],
                        ),
                    )
                nc.vector.tensor_mul(at2[mt][:, qsl], at2[mt][:, qsl], bc[:])

            def o_proj_block(qc):
                for tt in range(4 * qc, 4 * qc + 4):
                    ost = ostage_pool.tile([128, D], bf, tag="ost", name="ost")
                    for nb in range(2):
                        opsum = opsum_pool.tile(
                            [128, 512], f32, tag="o", name="opsum"
                        )
                        for hp in range(2):
                            nc.tensor.matmul(
                                opsum[:],
                                at2[hp][:, tt * 128 : (tt + 1) * 128],
                                wo_sb[:, hp, nb * 512 : (nb + 1) * 512],
                                start=(hp == 0),
                                stop=(hp == 1),
                            )
                        dst = ost[:, nb * 512 : (nb + 1) * 512]
                        if qc == 3:
                            nc.scalar.copy(out=dst, in_=opsum[:])
                        else:
                            nc.vector.tensor_copy(out=dst, in_=opsum[:])
                    nc.sync.dma_start(out=o_out[tt], in_=ost[:])

            # ============ emission schedule ============
            # all projection groups first (K before Q so the first S matmuls
            # unblock earliest); V tiles are woven into the early attention
            # iterations as PE filler while exp latency dominates
            qkt_group(wk_sb, kt_rot, 0, 0, raw_on_scalar=True)
            qkt_group(wk_sb, kt_rot, 1, 0, raw_on_scalar=True)
            qkt_group(wq_sb, qt_rot, 0, 0, raw_on_scalar=True)
            qkt_group(wq_sb, qt_rot, 1, 0, raw_on_scalar=True)
            qkt_group(wk_sb, kt_rot, 0, 1, raw_on_scalar=True)
            qkt_group(wk_sb, kt_rot, 1, 1, raw_on_scalar=True)
            qkt_group(wq_sb, qt_rot, 0, 1, raw_on_scalar=True)
            qkt_group(wq_sb, qt_rot, 1, 1, raw_on_scalar=True)

            fillers = {
                (0, 0): lambda: v_proj(range(0, 4), evict_on_scalar=True),
                (0, 1): lambda: v_proj(range(4, 8), evict_on_scalar=True),
                (1, 0): lambda: v_proj(range(8, 12), evict_on_scalar=False),
                (1, 1): lambda: v_proj(range(12, 16), evict_on_scalar=False),
            }

            iters = [(qc, mt) for qc in range(4) for mt in range(2)]
            pending = None  # (qc, mt, exps)
            for qc, mt in iters:
                exps = emit_s_exp(qc, mt)
                if pending is not None:
                    emit_pv_evict(*pending)
                    emit_normalize_mt(pending[0], pending[1])
                if (qc, mt) in fillers:
                    fillers[(qc, mt)]()
                if pending is not None and pending[1] == 1:
                    o_proj_block(pending[0])
                pending = (qc, mt, exps)
            emit_pv_evict(*pending)
            emit_normalize_mt(pending[0], pending[1])
            o_proj_block(pending[0])

    nc.compile()
    return nc


def get_program():
    global _PROG
    if _PROG is None:
        _PROG = _build_program()
    return _PROG


def _host_tables(bit_logits):
    """Replicate the reference fp32 cos/sin computation exactly (jax on CPU)."""
    import jax

    with jax.default_device(jax.devices("cpu")[0]):
        import jax.numpy as jnp

        basis = jnp.asarray(EULER_BASIS, dtype=jnp.float32)
        freqs = jax.nn.sigmoid(jnp.asarray(bit_logits, dtype=jnp.float32)) @ basis
        inv_freq = 2.0 ** (-(jnp.arange(0, DH, 2, dtype=jnp.float32) / DH))
        pos = jnp.arange(N, dtype=jnp.float32)
        theta = pos[None, :, None] * freqs[:, None, None] * inv_freq[None, None, :]
        cos = np.asarray(jnp.cos(theta))  # (H, N, 32)
        sin = np.asarray(jnp.sin(theta))
    return cos, sin


def _chunk_rows(a, p=128):
    """(R, C) -> (p, R//p, C); row r = kc*p + pp lands at [pp, kc]."""
    r, c = a.shape
    return np.ascontiguousarray(a.reshape(r // p, p, c).transpose(1, 0, 2))


def prepare_inputs(x, w_qkv, w_o, bit_logits):
    import ml_dtypes

    bf = ml_dtypes.bfloat16

    x = np.asarray(x, dtype=np.float32)
    w_qkv = np.asarray(w_qkv, dtype=np.float32)
    w_o = np.asarray(w_o, dtype=np.float32)
    cos, sin = _host_tables(np.asarray(bit_logits, dtype=np.float32))

    # de-interleave permutation within a head: evens then odds
    perm = np.concatenate([np.arange(0, DH, 2), np.arange(1, DH, 2)])

    wq_full = w_qkv.reshape(D, 3, H, DH)[:, 0]  # (D, H, DH)
    wk_full = w_qkv.reshape(D, 3, H, DH)[:, 1]
    wv_full = w_qkv.reshape(D, 3, H, DH)[:, 2]
    scale = 1.0 / math.sqrt(DH)

    # tri[krow, qcol] = 1 if qcol >= krow else 0, replicated 8x for the
    # strided diagonal mask
    tri = np.triu(np.ones((128, 128), dtype=np.float32))
    tri8 = np.broadcast_to(tri[:, None, :], (128, 8, 128)).copy()

    xT_by_batch = [
        _chunk_rows(np.ascontiguousarray(x[b].T)) for b in range(B)
    ]  # (128, KC, N)

    per_group = []
    for g in range(4):
        heads = range(4 * g, 4 * g + 4)
        wq_g = np.concatenate(
            [wq_full[:, h][:, perm] * scale for h in heads], axis=1
        )  # (D, 256)
        wk_g = np.concatenate([wk_full[:, h][:, perm] for h in heads], axis=1)
        wv_g = np.concatenate([wv_full[:, h] for h in heads], axis=1)
        wo_g = np.concatenate(
            [w_o.reshape(H, DH, D)[h] for h in heads], axis=0
        )  # (256, D)

        # rotation tables, layout (256 feats, N) -> (128, 2, N)
        ct = np.empty((DL, N), dtype=np.float32)
        st = np.empty((DL, N), dtype=np.float32)
        for hl, h in enumerate(heads):
            c = cos[h].T  # (32, N)
            s = sin[h].T
            ct[hl * DH : hl * DH + 32] = c
            ct[hl * DH + 32 : hl * DH + 64] = c
            st[hl * DH : hl * DH + 32] = -s
            st[hl * DH + 32 : hl * DH + 64] = s
        per_group.append(
            dict(
                wq=_chunk_rows(wq_g).astype(bf),
                wk=_chunk_rows(wk_g).astype(bf),
                wv=_chunk_rows(wv_g).astype(bf),
                wo=_chunk_rows(wo_g).astype(bf),
                ctab=_chunk_rows(ct).astype(bf),
                stab=_chunk_rows(st).astype(bf),
                tri8=tri8.astype(bf),
            )
        )

    in_maps = []
    for c in range(NCORES):
        b, g = c // 4, c % 4
        m = dict(per_group[g])
        m["xT"] = xT_by_batch[b].astype(bf)
        in_maps.append(m)
    return in_maps


def kernel(x, w_qkv, w_o, bit_logits, n_heads):
    global LAST_RESULTS
    from concourse.bass_utils import run_bass_kernel_spmd

    assert int(n_heads) == H
    nc = get_program()
    in_maps = prepare_inputs(x, w_qkv, w_o, bit_logits)
    res = run_bass_kernel_spmd(nc, in_maps, list(range(NCORES)))
    LAST_RESULTS = res
    out = np.zeros((B, N, D), dtype=np.float32)
    for c in range(NCORES):
        b = c // 4
        out[b] += res.results[c]["o_out"].reshape(N, D).astype(np.float32)
    return out


# revision 13
# speedup vs baseline: 1.2762x; 1.0769x over previous
"""EulerRotaryAttention Trainium2 kernel (bf16 matmul pipeline), v2.

Sharding: 8 cores = 2 (batch) x 4 (head groups of 4 heads).  Each core
computes the qkv projection for its heads, rotary attention, and a partial
o-projection; the host sums partials over the 4 head groups per batch.

Device dataflow (zero on-device transposes):
  - x^T arrives pre-transposed from the host as (d, n), bf16.
  - Q^T, K^T computed directly in (feat, tok) layout with the projection
    weights as the stationary matmul operand; fp32 PSUM accumulation.
  - RoPE rotation applied during PSUM eviction (host-permuted feature
    pairs 32 partitions apart; host-precomputed cos/sin tables).
  - S^T in (k, q) layout, causal tiles only, paired heads run as
    concurrent 64-row PE tiles; exp on ScalarE (no max subtraction).
  - PV: lhsT = [V | 1] so the accumulator yields A^T and the softmax
    denominators; reciprocal via the fast custom-DVE approx directly on
    the PSUM denominator row; partition broadcast via a DRAM bounce.
  - o-projection consumes A^T as lhsT; partial (n, d) written per core
    and summed on the host.

v2 schedule (vs. baseline): the whole kernel runs in ONE pool scope with
three PSUM pools (wide 2x2-bank slots shared by qkv-projection groups and
S-tiles, narrow 2x1-bank shared by V and PV, o 2x1-bank).  The second
token-half of the qkv projection and V tiles 8-15 are woven between the
early attention iterations, so ScalarE exp work starts ~30us earlier.
Eviction work is split Scalar/Vector by phase so exp is never queued
behind eviction copies.
"""

import math

import numpy as np

B, N, D, H = 2, 2048, 1024, 16
DH = D // H  # 64
HL = 4  # local heads per core
DL = HL * DH  # 256 local features
KC = D // 128  # 8 contraction chunks
NT = N // 128  # 16 token tiles
NCORES = 8

EULER_BASIS = (1.0, math.pi, math.e, math.pi * math.e, math.pi / math.e)

_PROG = None
LAST_RESULTS = None


def _build_program():
    import concourse.bass as bass
    import concourse.mybir as mybir
    import concourse.tile as tile
    from concourse import bacc

    f32 = mybir.dt.float32
    bf = mybir.dt.bfloat16
    AF = mybir.ActivationFunctionType

    nc = bacc.Bacc("TRN2", target_bir_lowering=False, num_devices=NCORES)

    xT = nc.declare_dram_parameter("xT", [128, KC, N], bf, isOutput=False)
    wq = nc.declare_dram_parameter("wq", [128, KC, DL], bf, isOutput=False)
    wk = nc.declare_dram_parameter("wk", [128, KC, DL], bf, isOutput=False)
    wv = nc.declare_dram_parameter("wv", [128, KC, DL], bf, isOutput=False)
    wo = nc.declare_dram_parameter("wo", [128, 2, D], bf, isOutput=False)
    ctab = nc.declare_dram_parameter("ctab", [128, 2, N], bf, isOutput=False)
    stab = nc.declare_dram_parameter("stab", [128, 2, N], bf, isOutput=False)
    tri8 = nc.declare_dram_parameter("tri8", [128, 8, 128], bf, isOutput=False)
    o_out = nc.declare_dram_parameter("o_out", [NT, 128, D], bf, isOutput=True)

    with tile.TileContext(nc) as tc:
        with (
            tc.tile_pool(name="persist", bufs=1) as persist,
            tc.tile_pool(name="exps_pool", bufs=2) as exps_pool,
            tc.tile_pool(name="rot_tmp", bufs=2) as rot_tmp,
            tc.tile_pool(name="norm_pool", bufs=2) as norm_pool,
            tc.tile_pool(name="bcast_pool", bufs=2) as bcast_pool,
            tc.tile_pool(name="dscr_pool", bufs=4, space="DRAM") as dscr_pool,
            tc.tile_pool(name="ostage_pool", bufs=3) as ostage_pool,
            # PSUM: 2x 2-bank slots (qkv-proj groups + S tiles), 2x 1-bank
            # (V + PV tiles), 2x 1-bank (o tiles) = 8 banks total
            tc.tile_pool(name="wide", bufs=2, space="PSUM") as wide,
            tc.tile_pool(name="narrow", bufs=2, space="PSUM") as narrow,
            tc.tile_pool(name="opsum_pool", bufs=2, space="PSUM") as opsum_pool,
        ):
            # rotated Q^T / K^T: (256 feats, N) as 2 x (128, N), bf16
            qt_rot = [
                persist.tile([128, N], bf, tag=f"qt{m}", name=f"qt{m}")
                for m in range(2)
            ]
            kt_rot = [
                persist.tile([128, N], bf, tag=f"kt{m}", name=f"kt{m}")
                for m in range(2)
            ]
            # V for all heads with appended ones column: (128, NT, HL, 65)
            vones = persist.tile([128, NT, HL, DH + 1], bf, tag="vones", name="vones")
            nc.vector.memset(vones[:, :, :, DH : DH + 1], 1.0)
            # A^T head pairs: (128, N) bf16
            at2 = [
                persist.tile([128, N], bf, tag=f"at{m}", name=f"at{m}")
                for m in range(2)
            ]
            xT_sb = persist.tile([128, KC, N], bf, tag="xT", name="xT_sb")
            wv_sb = persist.tile([128, KC, DL], bf, tag="wv", name="wv_sb")
            wq_sb = persist.tile([128, KC, DL], bf, tag="wq", name="wq_sb")
            wk_sb = persist.tile([128, KC, DL], bf, tag="wk", name="wk_sb")
            ctab_sb = persist.tile([128, 2, N], bf, tag="ctab", name="ctab_sb")
            stab_sb = persist.tile([128, 2, N], bf, tag="stab", name="stab_sb")
            tri8_sb = persist.tile([128, 8, 128], bf, tag="tri8", name="tri8_sb")
            wo_sb = persist.tile([128, 2, D], bf, tag="wo", name="wo_sb")

            # DMA order = first-use order.  xT split per (kc, token-half) so
            # the first projection group is not gated on the full 4MB.
            nc.sync.dma_start(out=wk_sb[:], in_=wk[:])
            for kc in range(KC):
                nc.sync.dma_start(
                    out=xT_sb[:, kc, 0:1024], in_=xT[:, kc, 0:1024]
                )
            nc.sync.dma_start(out=wq_sb[:], in_=wq[:])
            nc.sync.dma_start(out=ctab_sb[:], in_=ctab[:])
            nc.sync.dma_start(out=stab_sb[:], in_=stab[:])
            nc.sync.dma_start(out=tri8_sb[:], in_=tri8[:])
            for kc in range(KC):
                nc.sync.dma_start(
                    out=xT_sb[:, kc, 1024:2048], in_=xT[:, kc, 1024:2048]
                )
            nc.sync.dma_start(out=wv_sb[:], in_=wv[:])
            nc.sync.dma_start(out=wo_sb[:], in_=wo[:])

            def qkt_group(w_sb, rot, mt, nh, raw_on_scalar):
                """One (weight, head-pair, token-half) projection group with
                fused rotation eviction: rot = raw*ctab + swap32(raw)*stab."""
                nsl = slice(nh * 1024, (nh + 1) * 1024)
                psum = wide.tile([128, 1024], f32, tag="wide", name="qkpsum")
                for kc in range(KC):
                    for nq in range(2):
                        nc.tensor.matmul(
                            psum[:, nq * 512 : (nq + 1) * 512],
                            w_sb[:, kc, mt * 128 : (mt + 1) * 128],
                            xT_sb[
                                :,
                                kc,
                                nh * 1024 + nq * 512 : nh * 1024 + (nq + 1) * 512,
                            ],
                            start=(kc == 0),
                            stop=(kc == KC - 1),
                        )
                raw = rot_tmp.tile([128, 1024], bf, tag="raw", name="raw")
                if raw_on_scalar:
                    nc.scalar.copy(out=raw[:], in_=psum[:])
                else:
                    nc.vector.tensor_copy(out=raw[:], in_=psum[:])
                nc.vector.tensor_mul(rot[mt][:, nsl], raw[:], ctab_sb[:, mt, nsl])
                raws = rot_tmp.tile([128, 1024], bf, tag="rs", name="raws")
                for g in range(4):
                    s = g ^ 1
                    nc.vector.tensor_copy(
                        raws[g * 32 : (g + 1) * 32, :],
                        raw[s * 32 : (s + 1) * 32, :],
                    )
                tmp = rot_tmp.tile([128, 1024], bf, tag="rt", name="tmp")
                nc.vector.tensor_mul(tmp[:], raws[:], stab_sb[:, mt, nsl])
                nc.vector.tensor_add(rot[mt][:, nsl], rot[mt][:, nsl], tmp[:])

            def v_proj(tts, evict_on_scalar):
                for tt in tts:
                    vpsum = narrow.tile([128, DL], f32, tag="narrow", name="vpsum")
                    for kc in range(KC):
                        nc.tensor.matmul(
                            vpsum[:],
                            xT_sb[:, kc, tt * 128 : (tt + 1) * 128],
                            wv_sb[:, kc, :],
                            start=(kc == 0),
                            stop=(kc == KC - 1),
                        )
                    src = vpsum[:].rearrange("p (h d) -> p h d", h=HL)
                    if evict_on_scalar:
                        nc.scalar.copy(out=vones[:, tt, :, 0:DH], in_=src)
                    else:
                        nc.vector.tensor_copy(out=vones[:, tt, :, 0:DH], in_=src)

            rcp4s = {}

            def emit_s_exp(qc, mt):
                # paired-head S matmuls: the two heads of pair `mt` sit in
                # disjoint 64-partition halves of qt/kt -> concurrent PE
                # row-tiles; one ACT exp covers the pair
                nkt = 4 * qc + 4
                exps = exps_pool.tile([128, NT, 2, 512], bf, tag="e", name="exps")
                for kt in range(nkt):
                    j = kt - 4 * qc
                    jo = max(j, 0) * 128
                    spsum = wide.tile([128, 2, 512], f32, tag="wide", name="spsum")
                    for eo in range(2):
                        roff = eo * 64
                        nc.tensor.matmul(
                            spsum[:, eo, jo:512],
                            kt_rot[mt][roff : roff + 64, kt * 128 : (kt + 1) * 128],
                            qt_rot[mt][
                                roff : roff + 64,
                                qc * 512 + jo : qc * 512 + 512,
                            ],
                            start=True,
                            stop=True,
                        )
                    nc.scalar.activation(
                        exps[:, kt, :, jo:512], spsum[:, :, jo:512], AF.Exp
                    )
                # mask the 4 diagonal 128x128 subtiles of BOTH heads in one
                # strided op: (p, j, eo, c) -> exps[p, 4*qc+j, eo, j*128+c]
                sub = exps[:, 4 * qc, 0, :]
                diag = bass.AP(
                    tensor=sub.tensor,
                    offset=sub.offset,
                    ap=[list(sub.ap[0]), [1152, 4], [512, 2], [1, 128]],
                )
                t8 = tri8_sb[:, 0, :]
                trip = bass.AP(
                    tensor=t8.tensor,
                    offset=t8.offset,
                    ap=[list(t8.ap[0]), [256, 4], [128, 2], [1, 128]],
                )
                nc.vector.tensor_mul(diag, diag, trip)
                return exps

            def emit_pv_evict(qc, mt, exps):
                qsl = slice(qc * 512, (qc + 1) * 512)
                nkt = 4 * qc + 4
                if mt == 0:
                    # denominator rows live at partitions 0/32/64/96 (the
                    # only legal engine start partitions); pad rows are
                    # memset so the reciprocal input is fully initialized
                    rcp4s[qc] = norm_pool.tile([97, 512], f32, tag="rcp", name="rcp4")
                    nc.gpsimd.memset(rcp4s[qc][:], 1.0)
                rcp4 = rcp4s[qc]
                for eo in range(2):
                    h = 2 * mt + eo
                    roff = eo * 64
                    pv = narrow.tile([DH + 1, 512], f32, tag="narrow", name="pv")
                    for kt in range(nkt):
                        j = kt - 4 * qc
                        jo = max(j, 0) * 128
                        nc.tensor.matmul(
                            pv[:, jo:512],
                            vones[:, kt, h, :],
                            exps[:, kt, eo, jo:512],
                            start=(kt == 0),
                            stop=(kt == nkt - 1),
                        )
                    nc.vector.tensor_copy(
                        out=rcp4[32 * h : 32 * h + 1, :], in_=pv[DH : DH + 1, :]
                    )
                    # evict unnormalized A^T
                    if qc == 3 and mt == 1:
                        nc.scalar.copy(
                            out=at2[mt][roff : roff + DH, qsl], in_=pv[0:DH, :]
                        )
                    else:
                        nc.vector.tensor_copy(
                            out=at2[mt][roff : roff + DH, qsl], in_=pv[0:DH, :]
                        )

            rcp4os = {}

            def _bcast_mul(qc, mt):
                qsl = slice(qc * 512, (qc + 1) * 512)
                rcp4 = rcp4os[qc]
                bc = bcast_pool.tile([128, 512], f32, tag="bc", name="bc")
                # broadcast each head's reciprocal row across 64 partitions:
                # bounce through DRAM, then a step-0 partition DMA (legal
                # for DRAM sources only)
                for half in range(2):
                    row = rcp4[64 * mt + 32 * half : 64 * mt + 32 * half + 1, :]
                    rdram = dscr_pool.tile([1, 512], f32, tag="rd", name="rd")
                    nc.gpsimd.dma_start(out=rdram[:], in_=row)
                    rd = rdram[:]
                    nc.gpsimd.dma_start(
                        out=bc[64 * half : 64 * half + 64, :],
                        in_=bass.AP(
                            tensor=rd.tensor,
                            offset=rd.offset,
                            ap=[[0, 64], [1, 512]],
                        ),
                    )
                nc.vector.tensor_mul(at2[mt][:, qsl], at2[mt][:, qsl], bc[:])

            def emit_normalize_qc(qc):
                # qc 0-2: one batched reciprocal covering both head pairs
                rcp4os[qc] = norm_pool.tile([97, 512], f32, tag="rcpo", name="rcp4o")
                nc.vector.reciprocal(rcp4os[qc][:], rcp4s[qc][:])
                _bcast_mul(qc, 0)
                _bcast_mul(qc, 1)

            def emit_normalize_mt3(mt):
                # last q-chunk: per head-pair reciprocal so the tail chain
                # after the final PV is as short as possible
                if mt == 0:
                    rcp4os[3] = norm_pool.tile([97, 512], f32, tag="rcpo", name="rcp4o")
                sl = slice(64 * mt, 64 * mt + 33)
                nc.vector.reciprocal(rcp4os[3][sl, :], rcp4s[3][sl, :])
                _bcast_mul(3, mt)

            def o_proj_block(qc):
                for tt in range(4 * qc, 4 * qc + 4):
                    ost = ostage_pool.tile([128, D], bf, tag="ost", name="ost")
                    for nb in range(2):
                        opsum = opsum_pool.tile(
                            [128, 512], f32, tag="o", name="opsum"
                        )
                        for hp in range(2):
                            nc.tensor.matmul(
                                opsum[:],
                                at2[hp][:, tt * 128 : (tt + 1) * 128],
                                wo_sb[:, hp, nb * 512 : (nb + 1) * 512],
                                start=(hp == 0),
                                stop=(hp == 1),
                            )
                        dst = ost[:, nb * 512 : (nb + 1) * 512]
                        if qc == 3:
                            nc.scalar.copy(out=dst, in_=opsum[:])
                        else:
                            nc.vector.tensor_copy(out=dst, in_=opsum[:])
                    nc.sync.dma_start(out=o_out[tt], in_=ost[:])

            # ============ emission schedule ============
            # all projection groups first (K before Q so the first S matmuls
            # unblock earliest); V tiles are woven into the early attention
            # iterations as PE filler while exp latency dominates
            qkt_group(wk_sb, kt_rot, 0, 0, raw_on_scalar=True)
            qkt_group(wk_sb, kt_rot, 1, 0, raw_on_scalar=True)
            qkt_group(wq_sb, qt_rot, 0, 0, raw_on_scalar=True)
            qkt_group(wq_sb, qt_rot, 1, 0, raw_on_scalar=True)
            qkt_group(wk_sb, kt_rot, 0, 1, raw_on_scalar=True)
            qkt_group(wk_sb, kt_rot, 1, 1, raw_on_scalar=True)
            qkt_group(wq_sb, qt_rot, 0, 1, raw_on_scalar=True)
            qkt_group(wq_sb, qt_rot, 1, 1, raw_on_scalar=True)

            fillers = {
                (0, 0): lambda: v_proj(range(0, 4), evict_on_scalar=False),
                (0, 1): lambda: v_proj(range(4, 8), evict_on_scalar=False),
                (1, 0): lambda: v_proj(range(8, 12), evict_on_scalar=False),
                (1, 1): lambda: v_proj(range(12, 16), evict_on_scalar=False),
            }

            # o(qc) is emitted 1.5 iterations after PV(qc, 1) so the
            # normalize chain latency hides behind S/PV matmuls
            iters = [(qc, mt) for qc in range(4) for mt in range(2)]
            pending = None  # (qc, mt, exps)
            for qc, mt in iters:
                exps = emit_s_exp(qc, mt)
                if pending is not None:
                    emit_pv_evict(*pending)
                    if pending[0] < 3:
                        if pending[1] == 1:
                            emit_normalize_qc(pending[0])
                    else:
                        emit_normalize_mt3(pending[1])
                if (qc, mt) in fillers:
                    fillers[(qc, mt)]()
                if mt == 1 and qc >= 1:
                    o_proj_block(qc - 1)
                pending = (qc, mt, exps)
            emit_pv_evict(*pending)
            emit_normalize_mt3(1)
            o_proj_block(3)

    nc.compile()
    return nc


def get_program():
    global _PROG
    if _PROG is None:
        _PROG = _build_program()
    return _PROG


def _host_tables(bit_logits):
    """Replicate the reference fp32 cos/sin computation exactly (jax on CPU)."""
    import jax

    with jax.default_device(jax.devices("cpu")[0]):
        import jax.numpy as jnp

        basis = jnp.asarray(EULER_BASIS, dtype=jnp.float32)
        freqs = jax.nn.sigmoid(jnp.asarray(bit_logits, dtype=jnp.float32)) @ basis
        inv_freq = 2.0 ** (-(jnp.arange(0, DH, 2, dtype=jnp.float32) / DH))
        pos = jnp.arange(N, dtype=jnp.float32)
        theta = pos[None, :, None] * freqs[:, None, None] * inv_freq[None, None, :]
        cos = np.asarray(jnp.cos(theta))  # (H, N, 32)
        sin = np.asarray(jnp.sin(theta))
    return cos, sin


def _chunk_rows(a, p=128):
    """(R, C) -> (p, R//p, C); row r = kc*p + pp lands at [pp, kc]."""
    r, c = a.shape
    return np.ascontiguousarray(a.reshape(r // p, p, c).transpose(1, 0, 2))


def prepare_inputs(x, w_qkv, w_o, bit_logits):
    import ml_dtypes

    bf = ml_dtypes.bfloat16

    x = np.asarray(x, dtype=np.float32)
    w_qkv = np.asarray(w_qkv, dtype=np.float32)
    w_o = np.asarray(w_o, dtype=np.float32)
    cos, sin = _host_tables(np.asarray(bit_logits, dtype=np.float32))

    # de-interleave permutation within a head: evens then odds
    perm = np.concatenate([np.arange(0, DH, 2), np.arange(1, DH, 2)])

    wq_full = w_qkv.reshape(D, 3, H, DH)[:, 0]  # (D, H, DH)
    wk_full = w_qkv.reshape(D, 3, H, DH)[:, 1]
    wv_full = w_qkv.reshape(D, 3, H, DH)[:, 2]
    scale = 1.0 / math.sqrt(DH)

    # tri[krow, qcol] = 1 if qcol >= krow else 0, replicated 8x for the
    # strided diagonal mask
    tri = np.triu(np.ones((128, 128), dtype=np.float32))
    tri8 = np.broadcast_to(tri[:, None, :], (128, 8, 128)).copy()

    xT_by_batch = [
        _chunk_rows(np.ascontiguousarray(x[b].T)) for b in range(B)
    ]  # (128, KC, N)

    per_group = []
    for g in range(4):
        heads = range(4 * g, 4 * g + 4)
        wq_g = np.concatenate(
            [wq_full[:, h][:, perm] * scale for h in heads], axis=1
        )  # (D, 256)
        wk_g = np.concatenate([wk_full[:, h][:, perm] for h in heads], axis=1)
        wv_g = np.concatenate([wv_full[:, h] for h in heads], axis=1)
        wo_g = np.concatenate(
            [w_o.reshape(H, DH, D)[h] for h in heads], axis=0
        )  # (256, D)

        # rotation tables, layout (256 feats, N) -> (128, 2, N)
        ct = np.empty((DL, N), dtype=np.float32)
        st = np.empty((DL, N), dtype=np.float32)
        for hl, h in enumerate(heads):
            c = cos[h].T  # (32, N)
            s = sin[h].T
            ct[hl * DH : hl * DH + 32] = c
            ct[hl * DH + 32 : hl * DH + 64] = c
            st[hl * DH : hl * DH + 32] = -s
            st[hl * DH + 32 : hl * DH + 64] = s
        per_group.append(
            dict(
                wq=_chunk_rows(wq_g).astype(bf),
                wk=_chunk_rows(wk_g).astype(bf),
                wv=_chunk_rows(wv_g).astype(bf),
                wo=_chunk_rows(wo_g).astype(bf),
                ctab=_chunk_rows(ct).astype(bf),
                stab=_chunk_rows(st).astype(bf),
                tri8=tri8.astype(bf),
            )
        )

    in_maps = []
    for c in range(NCORES):
        b, g = c // 4, c % 4
        m = dict(per_group[g])
        m["xT"] = xT_by_batch[b].astype(bf)
        in_maps.append(m)
    return in_maps


def kernel(x, w_qkv, w_o, bit_logits, n_heads):
    global LAST_RESULTS
    from concourse.bass_utils import run_bass_kernel_spmd

    assert int(n_heads) == H
    nc = get_program()
    in_maps = prepare_inputs(x, w_qkv, w_o, bit_logits)
    res = run_bass_kernel_spmd(nc, in_maps, list(range(NCORES)))
    LAST_RESULTS = res
    out = np.zeros((B, N, D), dtype=np.float32)
    for c in range(NCORES):
        b = c // 4
        out[b] += res.results[c]["o_out"].reshape(N, D).astype(np.float32)
    return out
